# revision 35
# baseline (speedup 1.0000x reference)
"""Multi-head causal attention (B=2, S=2048, D=1024, H=16, hd=64) on 8 trn2
NeuronCores.

Sharding: core c -> batch b=c//4, head-group g=c%4 (4 heads = 256 contiguous
model dims). Each core computes q/k/v projections for its head group from the
full (transposed) batch-b input, runs causal attention for its 4 heads, and
applies its slice of the output projection, producing a partial [2048, 1024]
output (bf16). The host sums the 4 partials per batch in f32.

Pipelined structure: work is organized around 4 sequential 512-row
i-blocks. ~36 warmup matmuls on a scratch tile keep the PE array streaming
while the first inputs land, so the HAM clock boost (2.4GHz only after ~3us
of continuous high-utilization work; 1.2GHz otherwise) engages before real
work starts. Block 0's projections run standalone; thereafter the projection
units for block n+1 are INTERLEAVED into the attention streams of block n,
and output-projection tiles fire mid-way through the following stream (where
the Act queue is shallow, so their PSUM->SBUF copies release the shared psum
slots quickly). Within and across attention streams the AV matmuls lag the
QK matmuls by three units (lag-4 regresses; band-exp tiles need bufs=6),
hiding each unit's exp() latency behind later units' score matmuls. The
last stream normalizes its first 256 i-columns early (band B never touches
them) so the first two output tiles overlap the remaining work. The PE runs one continuous full-clock busy window from
~13us to the end of compute (measured: <3us of >250ns gaps).

DMA: the first two x blocks + all qkv weights are issued upfront on the
SP/Pool queues (the ~3.6MB the first two blocks need); x blocks 2/3 and the
wo tiles are issued from the Act queue mid-stream so their transfers can't
steal HBM bandwidth from earlier-needed data. y tiles stream out per block.

Matmul operands are bf16 (PE 1 cycle/row); accumulation is fp32 in PSUM.
Scores are computed transposed (S^T[j,i] = k^T.T @ q^T, contracting the
head's 64 dims via partition-offset slices of kT/qT) so the softmax
denominator comes free from the AV matmul via a ones-column appended to V,
and no transposes are needed anywhere. Causality is structural: only j<=i
blocks are computed; the 128x128 diagonal blocks get the (scaled) mask added.
exp() skips max-subtraction (scores are ~N(0,1); fp32 exp is safe and masked
entries underflow to exactly 0). The two diagonal-band score tiles share one
PSUM tile and one exp instruction per pair (Act instruction overhead is the
attention-phase limiter). All copies and DMA issues are kept OFF the Act
engine except the staggered mid-kernel loads.
"""

import sys

for p in ("/opt/trn_rl_repo", "/root/.axon_site/_ro/trn_rl_repo"):
    if p not in sys.path:
        sys.path.insert(0, p)

import ml_dtypes
import numpy as np

B, S, DIM, H, HD = 2, 2048, 1024, 16, 64
NCORES = 8
HG = 4  # heads per core
OG = HG * HD  # 256 output dims per core
NB = S // 512  # 4 i-blocks of 512
NJ = S // 128  # 16 j-tiles of 128

_CACHE = {}


def _build():
    import concourse.tile as tile
    from concourse import bacc, mybir

    f32 = mybir.dt.float32
    bf16 = mybir.dt.bfloat16
    Exp = mybir.ActivationFunctionType.Exp

    nc = bacc.Bacc("TRN2", target_bir_lowering=False, debug=False, num_devices=NCORES)

    # inputs, pre-packed on host (see _in_maps for layouts)
    xc = nc.dram_tensor("xc", [128, NB, 8, 512], bf16, kind="ExternalInput")
    wkq = nc.dram_tensor("wkq", [128, 4, 8, 128], bf16, kind="ExternalInput")
    wv2 = nc.dram_tensor("wv2", [128, 8, 256], bf16, kind="ExternalInput")
    woT = nc.dram_tensor("woT", [OG, DIM], bf16, kind="ExternalInput")
    cmask8 = nc.dram_tensor("cmask8", [128, 128], f32, kind="ExternalInput")
    y = nc.dram_tensor("y", [S, DIM], bf16, kind="ExternalOutput")

    woT_r = woT.ap().rearrange("(t p) e -> t p e", p=128)  # [2,128,1024]
    y_r = y.ap().rearrange("(t p) e -> t p e", p=128)  # [16,128,1024]

    with tile.TileContext(nc) as tc:
        with (
            tc.tile_pool(name="persist", bufs=1) as pp,
            tc.tile_pool(name="work", bufs=4) as wp,
            tc.tile_pool(name="psum", bufs=4, space="PSUM") as ps,
        ):
            # ---- persistent SBUF tiles -------------------------------------
            xsb = [
                pp.tile([128, 8, 512], bf16, tag=f"x{n}", name=f"x{n}")
                for n in range(NB)
            ]
            wkqt = pp.tile([128, 4, 8, 128], bf16, tag="wkq")
            wvt = pp.tile([128, 8, 256], bf16, tag="wv")
            wot = [pp.tile([128, DIM], bf16, tag=f"wo{i}", name=f"wo{i}") for i in range(2)]
            cm2 = pp.tile([128, 2, 128], f32, tag="cm2")
            kT = [pp.tile([128, S], bf16, tag=f"kT{i}", name=f"kT{i}") for i in range(2)]
            qT = [pp.tile([128, S], bf16, tag=f"qT{i}", name=f"qT{i}") for i in range(2)]
            vv = [pp.tile([128, HG, HD + 1], bf16, tag=f"vv{i}", name=f"vv{i}") for i in range(NJ)]
            zT = [pp.tile([128, S], bf16, tag=f"zT{i}", name=f"zT{i}") for i in range(2)]
            ones1 = pp.tile([1, 64], bf16, tag="ones1")
            scratch = pp.tile([128, 512], bf16, tag="scratch")

            sq, gq = nc.sync, nc.gpsimd

            # ---- upfront input DMAs (priority set: blocks 0/1 + weights) ---
            sq.dma_start(out=wkqt[:, 0], in_=wkq.ap()[:, 0])  # k m=0
            gq.dma_start(out=wkqt[:, 1], in_=wkq.ap()[:, 1])  # k m=1
            sq.dma_start(out=xsb[0][:, 0:4, :], in_=xc.ap()[:, 0, 0:4, :])
            gq.dma_start(out=xsb[0][:, 4:8, :], in_=xc.ap()[:, 0, 4:8, :])
            sq.dma_start(out=wkqt[:, 2], in_=wkq.ap()[:, 2])  # q m=0
            gq.dma_start(out=wkqt[:, 3], in_=wkq.ap()[:, 3])  # q m=1
            gq.dma_start(out=wvt, in_=wv2.ap())
            sq.dma_start(out=cm2[:, 0, :], in_=cmask8.ap())
            gq.dma_start(out=cm2[:, 1, :], in_=cmask8.ap())
            nc.vector.memset(ones1, 1.0)

            # ---- deferred softmax normalization ----------------------------
            pending = []

            def flush_norm():
                while pending:
                    pending.pop(0)()

            def mk_norm(m, po, n, psz, dn):
                def go():
                    psb2 = ps.tile([128, 1024], f32, tag="s2", bufs=3, name="psb2")
                    nc.tensor.matmul(
                        psb2[0:64, 0:512], ones1, dn, start=True, stop=True
                    )
                    rc = wp.tile([64, 512], f32, tag="rc", bufs=3, name="rc")
                    nc.vector.reciprocal_approx_fast(rc, psb2[0:64, 0:512])
                    nc.vector.tensor_mul(
                        zT[m][po : po + 64, n * 512 : (n + 1) * 512],
                        psz[0:64, :],
                        rc,
                    )
                return go

            # ---- projection units for one block (8 closures) ---------------
            def proj_units(n):
                xn = xsb[n]
                units = []

                def mk_kq(g):
                    # g: 0 = k m0, 1 = k m1, 2 = q m0, 3 = q m1
                    def go():
                        dst = kT[g % 2] if g < 2 else qT[g % 2]
                        acc2 = ps.tile([128, 1024], f32, tag="s2", bufs=3, name="acc2")
                        acc = acc2[:, 0:512]
                        for e in range(8):
                            nc.tensor.matmul(
                                acc,
                                wkqt[:, g, e, :],
                                xn[:, e, :],
                                start=(e == 0),
                                stop=(e == 7),
                            )
                        nc.vector.tensor_copy(dst[:, n * 512 : (n + 1) * 512], acc)
                    return go

                def mk_v(si):
                    def go():
                        s = 4 * n + si
                        acc2 = ps.tile([128, 1024], f32, tag="s2", bufs=3, name="acc2v")
                        acc = acc2[:, 0:256]
                        for e in range(8):
                            nc.tensor.matmul(
                                acc,
                                xn[:, e, si * 128 : (si + 1) * 128],
                                wvt[:, e, :],
                                start=(e == 0),
                                stop=(e == 7),
                            )
                        nc.vector.tensor_copy(
                            vv[s][:, :, 0:HD],
                            acc.rearrange("p (h d) -> p h d", h=HG),
                        )
                        nc.vector.memset(vv[s][:, :, HD : HD + 1], 1.0)
                    return go

                for g in range(4):
                    units.append(mk_kq(g))
                for si in range(4):
                    units.append(mk_v(si))
                return units

            # ---- attention stream (head h, i-block n) ----------------------
            # carry: closures handed from the previous stream (its last AV +
            # denominator copy), run after this stream's first QK so the PE
            # never waits on the previous stream's final exp.
            carry = []

            def run_carry():
                while carry:
                    carry.pop(0)()

            def stream(h, n, inject, mid_cb=None, final_tail=None):
                m, po = divmod(h, 2)
                po *= 64
                kTh = kT[m][po : po + 64, :]
                qTh = qT[m][po : po + 64, :]
                ib0 = n * 512
                psz = ps.tile([65, 512], f32, tag="z", bufs=2, name="psz")
                nplain = 4 * n
                units = []  # list of (qk_closure, av_closure)

                def mk_pair(jb):
                    ex_box = []

                    def qk():
                        pss = ps.tile([128, 1024], f32, tag="s2", bufs=3, name="pss")
                        for u in range(2):
                            nc.tensor.matmul(
                                pss[:, u * 512 : (u + 1) * 512],
                                kTh[:, (jb + u) * 128 : (jb + u + 1) * 128],
                                qTh[:, ib0 : ib0 + 512],
                                start=True,
                                stop=True,
                            )
                        ex = wp.tile([128, 1024], bf16, tag="ex", name="ex", bufs=6)
                        nc.scalar.activation(ex, pss, Exp, scale=0.125)
                        ex_box.append(ex)

                    def av():
                        ex = ex_box[0]
                        for u in range(2):
                            nc.tensor.matmul(
                                psz,
                                vv[jb + u][:, h, :],
                                ex[:, u * 512 : (u + 1) * 512],
                                start=(jb + u == 0),
                                stop=False,
                            )
                    return qk, av

                def mk_band_a():
                    jb = nplain
                    ex_box = []

                    def qk():
                        A = ps.tile([128, 1024], f32, tag="s2", bufs=3, name="A")
                        nc.tensor.matmul(
                            A[:, 0:512],
                            kTh[:, jb * 128 : (jb + 1) * 128],
                            qTh[:, ib0 : ib0 + 512],
                            start=True,
                            stop=True,
                        )
                        nc.tensor.matmul(
                            A[:, 512:896],
                            kTh[:, (jb + 1) * 128 : (jb + 2) * 128],
                            qTh[:, ib0 + 128 : ib0 + 512],
                            start=True,
                            stop=True,
                        )
                        Av = A.rearrange("p (t c) -> p t c", c=512)[:, :, 0:128]
                        nc.vector.tensor_add(Av, Av, cm2)
                        exA = wp.tile([128, 896], bf16, tag="exb", name="exA", bufs=6)
                        nc.scalar.activation(exA, A[:, 0:896], Exp, scale=0.125)
                        ex_box.append(exA)

                    def av():
                        exA = ex_box[0]
                        nc.tensor.matmul(
                            psz,
                            vv[jb][:, h, :],
                            exA[:, 0:512],
                            start=(jb == 0),
                            stop=False,
                        )
                        nc.tensor.matmul(
                            psz[:, 128:512],
                            vv[jb + 1][:, h, :],
                            exA[:, 512:896],
                            start=False,
                            stop=False,
                        )
                    return qk, av

                def mk_band_b():
                    jb = nplain + 2
                    ex_box = []

                    def qk():
                        Bt = ps.tile([128, 1024], f32, tag="s2", bufs=3, name="Bt")
                        nc.tensor.matmul(
                            Bt[:, 0:256],
                            kTh[:, jb * 128 : (jb + 1) * 128],
                            qTh[:, ib0 + 256 : ib0 + 512],
                            start=True,
                            stop=True,
                        )
                        nc.tensor.matmul(
                            Bt[:, 256:384],
                            kTh[:, (jb + 1) * 128 : (jb + 2) * 128],
                            qTh[:, ib0 + 384 : ib0 + 512],
                            start=True,
                            stop=True,
                        )
                        Bv = Bt[:, 0:512].rearrange("p (t c) -> p t c", c=256)[
                            :, :, 0:128
                        ]
                        nc.vector.tensor_add(Bv, Bv, cm2)
                        exB = wp.tile([128, 896], bf16, tag="exb", name="exB", bufs=6)
                        nc.scalar.activation(exB[:, 0:384], Bt[:, 0:384], Exp, scale=0.125)
                        ex_box.append(exB)

                    def av():
                        exB = ex_box[0]
                        nc.tensor.matmul(
                            psz[:, 256:512],
                            vv[jb][:, h, :],
                            exB[:, 0:256],
                            start=False,
                            stop=False,
                        )
                        nc.tensor.matmul(
                            psz[:, 384:512],
                            vv[jb + 1][:, h, :],
                            exB[:, 256:384],
                            start=False,
                            stop=True,
                        )
                    return qk, av

                for jb in range(0, nplain, 2):
                    units.append(mk_pair(jb))
                units.append(mk_band_a())
                units.append(mk_band_b())

                flush_at = min(3, len(units) - 1)
                mid_at = min(flush_at + 1, len(units) - 1)
                for ui, (qk, _) in enumerate(units):
                    qk()
                    if ui == 0:
                        run_carry()
                    elif ui >= 3:
                        units[ui - 3][1]()
                    inject()
                    if ui == flush_at:
                        flush_norm()
                    if ui == mid_at and mid_cb is not None:
                        mid_cb()

                if final_tail is None:
                    def tail():
                        for k in range(min(3, len(units)), 0, -1):
                            units[-k][1]()
                        dn = wp.tile([1, 512], bf16, tag="dn", bufs=3, name="dn")
                        nc.vector.tensor_copy(dn, psz[64:65, :])
                        pending.append(mk_norm(m, po, n, psz, dn))

                    carry.append(tail)
                    return

                # last stream of the kernel: psz[:, 0:256] is final after band
                # A's AV (band B only touches cols >= 256), so normalize that
                # half early and emit the first two output tiles while band
                # B's AV and the second half-norm still run.
                def half_norm(c0, c1):
                    dnh = wp.tile([1, 512], bf16, tag="dn", bufs=3, name="dnh")
                    nc.scalar.copy(dnh[:, c0:c1], psz[64:65, c0:c1])
                    psbh = ps.tile([128, 1024], f32, tag="s2", bufs=3, name="psbh")
                    nc.tensor.matmul(
                        psbh[0:64, c0:c1], ones1, dnh[:, c0:c1], start=True, stop=True
                    )
                    rch = wp.tile([64, 512], f32, tag="rc", bufs=3, name="rch")
                    nc.vector.reciprocal_approx_fast(rch[:, c0:c1], psbh[0:64, c0:c1])
                    nc.vector.tensor_mul(
                        zT[m][po : po + 64, ib0 + c0 : ib0 + c1],
                        psz[0:64, c0:c1],
                        rch[:, c0:c1],
                    )

                pre, mid2, post = final_tail
                for k in range(min(3, len(units)), 1, -1):
                    units[-k][1]()
                for cb in pre:
                    cb()
                half_norm(0, 256)
                for cb in mid2:
                    cb()
                units[-1][1]()
                half_norm(256, 512)
                for cb in post:
                    cb()

            # ---- output projection for one 128-row s-tile ------------------
            def oproj(s):
                psy2 = ps.tile([128, 1024], f32, tag="s2", bufs=3, name="psy2")
                for n2 in range(2):
                    for kk in range(2):
                        nc.tensor.matmul(
                            psy2[:, n2 * 512 : (n2 + 1) * 512],
                            zT[kk][:, s * 128 : (s + 1) * 128],
                            wot[kk][:, n2 * 512 : (n2 + 1) * 512],
                            start=(kk == 0),
                            stop=(kk == 1),
                        )
                ysb = wp.tile([128, DIM], bf16, tag="ysb", bufs=3, name="ysb")
                nc.vector.tensor_copy(ysb[:, 0:512], psy2[:, 0:512])
                sq.dma_start(out=y_r[s][:, 0:512], in_=ysb[:, 0:512])
                nc.vector.tensor_copy(ysb[:, 512:1024], psy2[:, 512:1024])
                sq.dma_start(out=y_r[s][:, 512:1024], in_=ysb[:, 512:1024])

            # ---- main pipelined loop ---------------------------------------
            def act_dma(dst, src):
                def go():
                    nc.scalar.dma_start(out=dst, in_=src)
                return go

            nc.vector.memset(scratch, 0.0)
            # warmup: keep the PE streaming while the first inputs land so
            # the HAM clock boost engages before real work starts (a cold PE
            # runs at half clock for its first ~3us of busy time)
            for w in range(9):
                wps = ps.tile([128, 1024], f32, tag="s2", bufs=3, name="wps")
                for r in range(4):
                    nc.tensor.matmul(
                        wps[:, 0:512], scratch[:, 0:128], scratch,
                        start=(r == 0), stop=(r == 3),
                    )

            for u in proj_units(0):
                u()

            opq = []
            for n in range(NB):
                # aux DMA issues (Act queue, delayed by its in-order position)
                # + projection units of block n+1, injected into this block's
                # attention streams
                nxt = []
                if n == 0:
                    nxt.append(act_dma(xsb[1][:, 0:4, :], xc.ap()[:, 1, 0:4, :]))
                    nxt.append(act_dma(xsb[1][:, 4:8, :], xc.ap()[:, 1, 4:8, :]))
                    nxt.append(act_dma(wot[0], woT_r[0]))
                    nxt.append(act_dma(wot[1], woT_r[1]))
                if n + 2 < NB:
                    nxt.append(
                        act_dma(xsb[n + 2], xc.ap()[:, n + 2])
                    )
                if n + 1 < NB:
                    nxt.extend(proj_units(n + 1))
                n_units = 4 * (2 * n + 2)
                frac = [0.0]

                def inject(nxt=nxt, n_units=n_units, frac=frac, total=len(nxt)):
                    frac[0] += total / n_units
                    while nxt and frac[0] >= 1.0:
                        frac[0] -= 1.0
                        nxt.pop(0)()

                for h in range(HG):
                    mid = None
                    if opq and (n > 1 or h > 0):
                        s_out = opq.pop(0)
                        mid = (lambda s_out=s_out: oproj(s_out))
                    ft = None
                    if n == NB - 1 and h == HG - 1:
                        ft = (
                            [lambda: oproj(11)],
                            [lambda: oproj(12), lambda: oproj(13)],
                            [lambda: oproj(14), lambda: oproj(15)],
                        )
                    stream(h, n, inject, mid, ft)
                    if n > 0:
                        opq.append(4 * (n - 1) + h)
                while nxt:
                    nxt.pop(0)()

    nc.compile()
    return nc


def _get_nc():
    if "nc" not in _CACHE:
        _CACHE["nc"] = _build()
    return _CACHE["nc"]


def _in_maps(x, mask, wq, wk, wv, wo):
    bf = ml_dtypes.bfloat16
    cm8 = np.ascontiguousarray(8.0 * np.asarray(mask)[0, 0, :128, :128].T, np.float32)
    maps = []
    for c in range(NCORES):
        b, g = divmod(c, HG)
        sl = slice(OG * g, OG * (g + 1))
        # xc[p, n, e, c] = x[b][512n+c, 128e+p]
        xT = np.asarray(x)[b].T.astype(bf)  # [1024, 2048]
        xcm = np.ascontiguousarray(
            xT.reshape(8, 128, NB, 512).transpose(1, 2, 0, 3)
        )
        # wkq[p, g, e, :]: g0/g1 = wk m-chunks, g2/g3 = wq m-chunks
        wkT = np.asarray(wk)[sl, :].T.astype(bf)  # [1024, 256]
        wqT = np.asarray(wq)[sl, :].T.astype(bf)
        wkqm = np.ascontiguousarray(
            np.stack(
                [wkT[:, 0:128], wkT[:, 128:256], wqT[:, 0:128], wqT[:, 128:256]],
                axis=0,
            )
            .reshape(4, 8, 128, 128)
            .transpose(2, 0, 1, 3)
        )
        wvT = np.asarray(wv)[sl, :].T.astype(bf)  # [1024, 256]
        wv2m = np.ascontiguousarray(wvT.reshape(8, 128, 256).transpose(1, 0, 2))
        maps.append(
            {
                "xc": xcm,
                "wkq": wkqm,
                "wv2": wv2m,
                "woT": np.ascontiguousarray(np.asarray(wo)[:, sl].T).astype(bf),
                "cmask8": cm8,
            }
        )
    return maps


def _combine(results):
    y = np.zeros((B, S, DIM), np.float32)
    for c in range(NCORES):
        y[c // HG] += results[c]["y"].astype(np.float32)
    return y


def kernel(x, mask, wq, wk, wv, wo, **run_kwargs):
    from concourse.bass_utils import run_bass_kernel_spmd

    nc = _get_nc()
    res = run_bass_kernel_spmd(
        nc, _in_maps(x, mask, wq, wk, wv, wo), core_ids=list(range(NCORES)),
        **run_kwargs,
    )
    out = _combine(res.results)
    if run_kwargs:
        _CACHE["last_result"] = res
    return out


# revision 36
# speedup vs baseline: 1.0113x; 1.0113x over previous
"""Multi-head causal attention (B=2, S=2048, D=1024, H=16, hd=64) on 8 trn2
NeuronCores.

Sharding: core c -> batch b=c//4, head-group g=c%4 (4 heads = 256 contiguous
model dims). Each core computes q/k/v projections for its head group from the
full (transposed) batch-b input, runs causal attention for its 4 heads, and
applies its slice of the output projection, producing a partial [2048, 1024]
output (bf16). The host sums the 4 partials per batch in f32.

Pipelined structure: work is organized around 4 sequential 512-row
i-blocks. ~36 warmup matmuls on a scratch tile keep the PE array streaming
while the first inputs land, so the HAM clock boost (2.4GHz only after ~3us
of continuous high-utilization work; 1.2GHz otherwise) engages before real
work starts. Block 0's projections run standalone; thereafter the projection
units for block n+1 are INTERLEAVED into the attention streams of block n,
and output-projection tiles fire mid-way through the following stream (where
the Act queue is shallow, so their PSUM->SBUF copies release the shared psum
slots quickly). Within and across attention streams the AV matmuls lag the
QK matmuls by three units (lag-4 regresses; band-exp tiles need bufs=6),
hiding each unit's exp() latency behind later units' score matmuls. The
last stream normalizes its first 256 i-columns early (band B never touches
them) so the first two output tiles overlap the remaining work. The PE runs one continuous full-clock busy window from
~13us to the end of compute (measured: <3us of >250ns gaps).

DMA: the first two x blocks + all qkv weights are issued upfront on the
SP/Pool queues (the ~3.6MB the first two blocks need); x blocks 2/3 and the
wo tiles are issued from the Act queue mid-stream so their transfers can't
steal HBM bandwidth from earlier-needed data. y tiles stream out per block.

Matmul operands are bf16 (PE 1 cycle/row); accumulation is fp32 in PSUM.
Scores are computed transposed (S^T[j,i] = k^T.T @ q^T, contracting the
head's 64 dims via partition-offset slices of kT/qT) so the softmax
denominator comes free from the AV matmul via a ones-column appended to V,
and no transposes are needed anywhere. Causality is structural: only j<=i
blocks are computed; the 128x128 diagonal blocks get the (scaled) mask added.
exp() skips max-subtraction (scores are ~N(0,1); fp32 exp is safe and masked
entries underflow to exactly 0). The two diagonal-band score tiles share one
PSUM tile and one exp instruction per pair (Act instruction overhead is the
attention-phase limiter). All copies and DMA issues are kept OFF the Act
engine except the staggered mid-kernel loads.
"""

import sys

for p in ("/opt/trn_rl_repo", "/root/.axon_site/_ro/trn_rl_repo"):
    if p not in sys.path:
        sys.path.insert(0, p)

import ml_dtypes
import numpy as np

B, S, DIM, H, HD = 2, 2048, 1024, 16, 64
NCORES = 8
HG = 4  # heads per core
OG = HG * HD  # 256 output dims per core
NB = S // 512  # 4 i-blocks of 512
NJ = S // 128  # 16 j-tiles of 128

_CACHE = {}


def _build():
    import concourse.tile as tile
    from concourse import bacc, mybir

    f32 = mybir.dt.float32
    bf16 = mybir.dt.bfloat16
    Exp = mybir.ActivationFunctionType.Exp

    nc = bacc.Bacc("TRN2", target_bir_lowering=False, debug=False, num_devices=NCORES)

    # inputs, pre-packed on host (see _in_maps for layouts)
    xc = nc.dram_tensor("xc", [128, NB, 8, 512], bf16, kind="ExternalInput")
    wkq = nc.dram_tensor("wkq", [128, 4, 8, 128], bf16, kind="ExternalInput")
    wv2 = nc.dram_tensor("wv2", [128, 8, 256], bf16, kind="ExternalInput")
    woT = nc.dram_tensor("woT", [OG, DIM], bf16, kind="ExternalInput")
    cmask8 = nc.dram_tensor("cmask8", [128, 128], f32, kind="ExternalInput")
    y = nc.dram_tensor("y", [S, DIM], bf16, kind="ExternalOutput")

    woT_r = woT.ap().rearrange("(t p) e -> t p e", p=128)  # [2,128,1024]
    y_r = y.ap().rearrange("(t p) e -> t p e", p=128)  # [16,128,1024]

    with tile.TileContext(nc) as tc:
        with (
            tc.tile_pool(name="persist", bufs=1) as pp,
            tc.tile_pool(name="work", bufs=4) as wp,
            tc.tile_pool(name="psum", bufs=4, space="PSUM") as ps,
        ):
            # ---- persistent SBUF tiles -------------------------------------
            xsb = [
                pp.tile([128, 8, 512], bf16, tag=f"x{n}", name=f"x{n}")
                for n in range(NB)
            ]
            wkqt = pp.tile([128, 4, 8, 128], bf16, tag="wkq")
            wvt = pp.tile([128, 8, 256], bf16, tag="wv")
            wot = [pp.tile([128, DIM], bf16, tag=f"wo{i}", name=f"wo{i}") for i in range(2)]
            cm2 = pp.tile([128, 2, 128], f32, tag="cm2")
            kT = [pp.tile([128, S], bf16, tag=f"kT{i}", name=f"kT{i}") for i in range(2)]
            qT = [pp.tile([128, S], bf16, tag=f"qT{i}", name=f"qT{i}") for i in range(2)]
            vv = [pp.tile([128, HG, HD + 1], bf16, tag=f"vv{i}", name=f"vv{i}") for i in range(NJ)]
            zT = [pp.tile([128, S], bf16, tag=f"zT{i}", name=f"zT{i}") for i in range(2)]
            ones1 = pp.tile([1, 64], bf16, tag="ones1")
            scratch = pp.tile([128, 512], bf16, tag="scratch")

            sq, gq = nc.sync, nc.gpsimd

            # ---- upfront input DMAs (priority set: blocks 0/1 + weights) ---
            sq.dma_start(out=wkqt[:, 0], in_=wkq.ap()[:, 0])  # k m=0
            gq.dma_start(out=wkqt[:, 1], in_=wkq.ap()[:, 1])  # k m=1
            sq.dma_start(out=xsb[0][:, 0:4, :], in_=xc.ap()[:, 0, 0:4, :])
            gq.dma_start(out=xsb[0][:, 4:8, :], in_=xc.ap()[:, 0, 4:8, :])
            sq.dma_start(out=wkqt[:, 2], in_=wkq.ap()[:, 2])  # q m=0
            gq.dma_start(out=wkqt[:, 3], in_=wkq.ap()[:, 3])  # q m=1
            gq.dma_start(out=wvt, in_=wv2.ap())
            sq.dma_start(out=cm2[:, 0, :], in_=cmask8.ap())
            gq.dma_start(out=cm2[:, 1, :], in_=cmask8.ap())
            nc.vector.memset(ones1, 1.0)

            # ---- deferred softmax normalization ----------------------------
            pending = []

            def flush_norm():
                while pending:
                    pending.pop(0)()

            def mk_norm(m, po, n, psz, dn):
                def go():
                    psb2 = ps.tile([128, 1024], f32, tag="s2", bufs=3, name="psb2")
                    nc.tensor.matmul(
                        psb2[0:64, 0:512], ones1, dn, start=True, stop=True
                    )
                    rc = wp.tile([64, 512], f32, tag="rc", bufs=3, name="rc")
                    nc.vector.reciprocal_approx_fast(rc, psb2[0:64, 0:512])
                    nc.vector.tensor_mul(
                        zT[m][po : po + 64, n * 512 : (n + 1) * 512],
                        psz[0:64, :],
                        rc,
                    )
                return go

            # ---- projection units for one block (8 closures) ---------------
            def proj_units(n):
                xn = xsb[n]
                units = []

                def mk_kq(g):
                    # g: 0 = k m0, 1 = k m1, 2 = q m0, 3 = q m1
                    def go():
                        dst = kT[g % 2] if g < 2 else qT[g % 2]
                        acc2 = ps.tile([128, 1024], f32, tag="s2", bufs=3, name="acc2")
                        acc = acc2[:, 0:512]
                        for e in range(8):
                            nc.tensor.matmul(
                                acc,
                                wkqt[:, g, e, :],
                                xn[:, e, :],
                                start=(e == 0),
                                stop=(e == 7),
                            )
                        nc.vector.tensor_copy(dst[:, n * 512 : (n + 1) * 512], acc)
                    return go

                def mk_v(si):
                    def go():
                        s = 4 * n + si
                        acc2 = ps.tile([128, 1024], f32, tag="s2", bufs=3, name="acc2v")
                        acc = acc2[:, 0:256]
                        for e in range(8):
                            nc.tensor.matmul(
                                acc,
                                xn[:, e, si * 128 : (si + 1) * 128],
                                wvt[:, e, :],
                                start=(e == 0),
                                stop=(e == 7),
                            )
                        nc.vector.tensor_copy(
                            vv[s][:, :, 0:HD],
                            acc.rearrange("p (h d) -> p h d", h=HG),
                        )
                        nc.vector.memset(vv[s][:, :, HD : HD + 1], 1.0)
                    return go

                for g in range(4):
                    units.append(mk_kq(g))
                for si in range(4):
                    units.append(mk_v(si))
                return units

            # ---- attention stream (head h, i-block n) ----------------------
            # carry: closures handed from the previous stream (its last AV +
            # denominator copy), run after this stream's first QK so the PE
            # never waits on the previous stream's final exp.
            carry = []

            def run_carry():
                while carry:
                    carry.pop(0)()

            def stream(h, n, inject, mid_cb=None, final_tail=None):
                m, po = divmod(h, 2)
                po *= 64
                kTh = kT[m][po : po + 64, :]
                qTh = qT[m][po : po + 64, :]
                ib0 = n * 512
                psz = ps.tile([65, 512], f32, tag="z", bufs=2, name="psz")
                nplain = 4 * n
                units = []  # list of (qk_closure, av_closure)

                def mk_pair(jb):
                    ex_box = []

                    def qk():
                        pss = ps.tile([128, 1024], f32, tag="s2", bufs=3, name="pss")
                        for u in range(2):
                            nc.tensor.matmul(
                                pss[:, u * 512 : (u + 1) * 512],
                                kTh[:, (jb + u) * 128 : (jb + u + 1) * 128],
                                qTh[:, ib0 : ib0 + 512],
                                start=True,
                                stop=True,
                            )
                        ex = wp.tile([128, 1024], bf16, tag="ex", name="ex", bufs=6)
                        nc.scalar.activation(ex, pss, Exp, scale=0.125)
                        ex_box.append(ex)

                    def av():
                        ex = ex_box[0]
                        for u in range(2):
                            nc.tensor.matmul(
                                psz,
                                vv[jb + u][:, h, :],
                                ex[:, u * 512 : (u + 1) * 512],
                                start=(jb + u == 0),
                                stop=False,
                            )
                    return qk, av

                def mk_band_a():
                    jb = nplain
                    ex_box = []

                    def qk():
                        A = ps.tile([128, 1024], f32, tag="s2", bufs=3, name="A")
                        nc.tensor.matmul(
                            A[:, 0:512],
                            kTh[:, jb * 128 : (jb + 1) * 128],
                            qTh[:, ib0 : ib0 + 512],
                            start=True,
                            stop=True,
                        )
                        nc.tensor.matmul(
                            A[:, 512:896],
                            kTh[:, (jb + 1) * 128 : (jb + 2) * 128],
                            qTh[:, ib0 + 128 : ib0 + 512],
                            start=True,
                            stop=True,
                        )
                        Av = A.rearrange("p (t c) -> p t c", c=512)[:, :, 0:128]
                        nc.vector.tensor_add(Av, Av, cm2)
                        exA = wp.tile([128, 896], bf16, tag="exb", name="exA", bufs=6)
                        nc.scalar.activation(exA, A[:, 0:896], Exp, scale=0.125)
                        ex_box.append(exA)

                    def av():
                        exA = ex_box[0]
                        nc.tensor.matmul(
                            psz,
                            vv[jb][:, h, :],
                            exA[:, 0:512],
                            start=(jb == 0),
                            stop=False,
                        )
                        nc.tensor.matmul(
                            psz[:, 128:512],
                            vv[jb + 1][:, h, :],
                            exA[:, 512:896],
                            start=False,
                            stop=False,
                        )
                    return qk, av

                def mk_band_b():
                    jb = nplain + 2
                    ex_box = []

                    def qk():
                        Bt = ps.tile([128, 1024], f32, tag="s2", bufs=3, name="Bt")
                        nc.tensor.matmul(
                            Bt[:, 0:256],
                            kTh[:, jb * 128 : (jb + 1) * 128],
                            qTh[:, ib0 + 256 : ib0 + 512],
                            start=True,
                            stop=True,
                        )
                        nc.tensor.matmul(
                            Bt[:, 256:384],
                            kTh[:, (jb + 1) * 128 : (jb + 2) * 128],
                            qTh[:, ib0 + 384 : ib0 + 512],
                            start=True,
                            stop=True,
                        )
                        Bv = Bt[:, 0:512].rearrange("p (t c) -> p t c", c=256)[
                            :, :, 0:128
                        ]
                        nc.vector.tensor_add(Bv, Bv, cm2)
                        exB = wp.tile([128, 896], bf16, tag="exb", name="exB", bufs=6)
                        nc.scalar.activation(exB[:, 0:384], Bt[:, 0:384], Exp, scale=0.125)
                        ex_box.append(exB)

                    def av():
                        exB = ex_box[0]
                        nc.tensor.matmul(
                            psz[:, 256:512],
                            vv[jb][:, h, :],
                            exB[:, 0:256],
                            start=False,
                            stop=False,
                        )
                        nc.tensor.matmul(
                            psz[:, 384:512],
                            vv[jb + 1][:, h, :],
                            exB[:, 256:384],
                            start=False,
                            stop=True,
                        )
                    return qk, av

                for jb in range(0, nplain, 2):
                    units.append(mk_pair(jb))
                units.append(mk_band_a())
                units.append(mk_band_b())

                flush_at = min(3, len(units) - 1)
                mid_at = min(flush_at + 1, len(units) - 1)
                for ui, (qk, _) in enumerate(units):
                    qk()
                    if ui == 0:
                        run_carry()
                    elif ui >= 3:
                        units[ui - 3][1]()
                    inject()
                    if ui == flush_at:
                        flush_norm()
                    if ui == mid_at and mid_cb is not None:
                        mid_cb()

                if final_tail is None:
                    def tail():
                        for k in range(min(3, len(units)), 0, -1):
                            units[-k][1]()
                        dn = wp.tile([1, 512], bf16, tag="dn", bufs=3, name="dn")
                        nc.vector.tensor_copy(dn, psz[64:65, :])
                        pending.append(mk_norm(m, po, n, psz, dn))

                    carry.append(tail)
                    return

                # last stream of the kernel: psz[:, 0:256] is final after band
                # A's AV (band B only touches cols >= 256), so normalize that
                # half early and emit the first two output tiles while band
                # B's AV and the second half-norm still run.
                def half_norm(c0, c1):
                    dnh = wp.tile([1, 512], bf16, tag="dn", bufs=3, name="dnh")
                    nc.scalar.copy(dnh[:, c0:c1], psz[64:65, c0:c1])
                    psbh = ps.tile([128, 1024], f32, tag="s2", bufs=3, name="psbh")
                    nc.tensor.matmul(
                        psbh[0:64, c0:c1], ones1, dnh[:, c0:c1], start=True, stop=True
                    )
                    rch = wp.tile([64, 512], f32, tag="rc", bufs=3, name="rch")
                    nc.vector.reciprocal_approx_fast(rch[:, c0:c1], psbh[0:64, c0:c1])
                    nc.vector.tensor_mul(
                        zT[m][po : po + 64, ib0 + c0 : ib0 + c1],
                        psz[0:64, c0:c1],
                        rch[:, c0:c1],
                    )

                pre, mid2, post = final_tail
                for k in range(min(3, len(units)), 1, -1):
                    units[-k][1]()
                for cb in pre:
                    cb()
                half_norm(0, 256)
                for cb in mid2:
                    cb()
                units[-1][1]()
                half_norm(256, 512)
                for cb in post:
                    cb()

            # ---- output projection for one 128-row s-tile ------------------
            def oproj(s):
                psy2 = ps.tile([128, 1024], f32, tag="s2", bufs=3, name="psy2")
                for n2 in range(2):
                    for kk in range(2):
                        nc.tensor.matmul(
                            psy2[:, n2 * 512 : (n2 + 1) * 512],
                            zT[kk][:, s * 128 : (s + 1) * 128],
                            wot[kk][:, n2 * 512 : (n2 + 1) * 512],
                            start=(kk == 0),
                            stop=(kk == 1),
                        )
                ysb = wp.tile([128, DIM], bf16, tag="ysb", bufs=3, name="ysb")
                if s < 8:
                    nc.scalar.copy(ysb[:, 0:512], psy2[:, 0:512])
                else:
                    nc.vector.tensor_copy(ysb[:, 0:512], psy2[:, 0:512])
                sq.dma_start(out=y_r[s][:, 0:512], in_=ysb[:, 0:512])
                nc.vector.tensor_copy(ysb[:, 512:1024], psy2[:, 512:1024])
                sq.dma_start(out=y_r[s][:, 512:1024], in_=ysb[:, 512:1024])

            # ---- main pipelined loop ---------------------------------------
            def act_dma(dst, src):
                def go():
                    nc.scalar.dma_start(out=dst, in_=src)
                return go

            nc.vector.memset(scratch, 0.0)
            # warmup: keep the PE streaming while the first inputs land so
            # the HAM clock boost engages before real work starts (a cold PE
            # runs at half clock for its first ~3us of busy time)
            for w in range(9):
                wps = ps.tile([128, 1024], f32, tag="s2", bufs=3, name="wps")
                for r in range(4):
                    nc.tensor.matmul(
                        wps[:, 0:512], scratch[:, 0:128], scratch,
                        start=(r == 0), stop=(r == 3),
                    )

            for u in proj_units(0):
                u()

            opq = []
            for n in range(NB):
                # aux DMA issues (Act queue, delayed by its in-order position)
                # + projection units of block n+1, injected into this block's
                # attention streams
                nxt = []
                if n == 0:
                    nxt.append(act_dma(xsb[1][:, 0:4, :], xc.ap()[:, 1, 0:4, :]))
                    nxt.append(act_dma(xsb[1][:, 4:8, :], xc.ap()[:, 1, 4:8, :]))
                    nxt.append(act_dma(wot[0], woT_r[0]))
                    nxt.append(act_dma(wot[1], woT_r[1]))
                if n + 2 < NB:
                    nxt.append(
                        act_dma(xsb[n + 2], xc.ap()[:, n + 2])
                    )
                if n + 1 < NB:
                    nxt.extend(proj_units(n + 1))
                n_units = 4 * (2 * n + 2)
                frac = [0.0]

                def inject(nxt=nxt, n_units=n_units, frac=frac, total=len(nxt)):
                    frac[0] += total / n_units
                    while nxt and frac[0] >= 1.0:
                        frac[0] -= 1.0
                        nxt.pop(0)()

                for h in range(HG):
                    mid = None
                    if opq and (n > 1 or h > 0):
                        s_out = opq.pop(0)
                        mid = (lambda s_out=s_out: oproj(s_out))
                    ft = None
                    if n == NB - 1 and h == HG - 1:
                        ft = (
                            [lambda: oproj(11)],
                            [lambda: oproj(12), lambda: oproj(13)],
                            [lambda: oproj(14), lambda: oproj(15)],
                        )
                    stream(h, n, inject, mid, ft)
                    if n > 0:
                        opq.append(4 * (n - 1) + h)
                while nxt:
                    nxt.pop(0)()

    nc.compile()
    return nc


def _get_nc():
    if "nc" not in _CACHE:
        _CACHE["nc"] = _build()
    return _CACHE["nc"]


def _in_maps(x, mask, wq, wk, wv, wo):
    bf = ml_dtypes.bfloat16
    cm8 = np.ascontiguousarray(8.0 * np.asarray(mask)[0, 0, :128, :128].T, np.float32)
    maps = []
    for c in range(NCORES):
        b, g = divmod(c, HG)
        sl = slice(OG * g, OG * (g + 1))
        # xc[p, n, e, c] = x[b][512n+c, 128e+p]
        xT = np.asarray(x)[b].T.astype(bf)  # [1024, 2048]
        xcm = np.ascontiguousarray(
            xT.reshape(8, 128, NB, 512).transpose(1, 2, 0, 3)
        )
        # wkq[p, g, e, :]: g0/g1 = wk m-chunks, g2/g3 = wq m-chunks
        wkT = np.asarray(wk)[sl, :].T.astype(bf)  # [1024, 256]
        wqT = np.asarray(wq)[sl, :].T.astype(bf)
        wkqm = np.ascontiguousarray(
            np.stack(
                [wkT[:, 0:128], wkT[:, 128:256], wqT[:, 0:128], wqT[:, 128:256]],
                axis=0,
            )
            .reshape(4, 8, 128, 128)
            .transpose(2, 0, 1, 3)
        )
        wvT = np.asarray(wv)[sl, :].T.astype(bf)  # [1024, 256]
        wv2m = np.ascontiguousarray(wvT.reshape(8, 128, 256).transpose(1, 0, 2))
        maps.append(
            {
                "xc": xcm,
                "wkq": wkqm,
                "wv2": wv2m,
                "woT": np.ascontiguousarray(np.asarray(wo)[:, sl].T).astype(bf),
                "cmask8": cm8,
            }
        )
    return maps


def _combine(results):
    y = np.zeros((B, S, DIM), np.float32)
    for c in range(NCORES):
        y[c // HG] += results[c]["y"].astype(np.float32)
    return y


def kernel(x, mask, wq, wk, wv, wo, **run_kwargs):
    from concourse.bass_utils import run_bass_kernel_spmd

    nc = _get_nc()
    res = run_bass_kernel_spmd(
        nc, _in_maps(x, mask, wq, wk, wv, wo), core_ids=list(range(NCORES)),
        **run_kwargs,
    )
    out = _combine(res.results)
    if run_kwargs:
        _CACHE["last_result"] = res
    return out


# revision 37
# speedup vs baseline: 1.0133x; 1.0020x over previous
"""Multi-head causal attention (B=2, S=2048, D=1024, H=16, hd=64) on 8 trn2
NeuronCores.

Sharding: core c -> batch b=c//4, head-group g=c%4 (4 heads = 256 contiguous
model dims). Each core computes q/k/v projections for its head group from the
full (transposed) batch-b input, runs causal attention for its 4 heads, and
applies its slice of the output projection, producing a partial [2048, 1024]
output (bf16). The host sums the 4 partials per batch in f32.

Pipelined structure: work is organized around 4 sequential 512-row
i-blocks. ~36 warmup matmuls on a scratch tile keep the PE array streaming
while the first inputs land, so the HAM clock boost (2.4GHz only after ~3us
of continuous high-utilization work; 1.2GHz otherwise) engages before real
work starts. Block 0's projections run standalone; thereafter the projection
units for block n+1 are INTERLEAVED into the attention streams of block n,
and output-projection tiles fire mid-way through the following stream (where
the Act queue is shallow, so their PSUM->SBUF copies release the shared psum
slots quickly). Within and across attention streams the AV matmuls lag the
QK matmuls by three units (lag-4 regresses; band-exp tiles need bufs=6),
hiding each unit's exp() latency behind later units' score matmuls. The
last stream normalizes its first 256 i-columns early (band B never touches
them) so the first two output tiles overlap the remaining work. The PE runs one continuous full-clock busy window from
~13us to the end of compute (measured: <3us of >250ns gaps).

DMA: the first two x blocks + all qkv weights are issued upfront on the
SP/Pool queues (the ~3.6MB the first two blocks need); x blocks 2/3 and the
wo tiles are issued from the Act queue mid-stream so their transfers can't
steal HBM bandwidth from earlier-needed data. y tiles stream out per block.

Matmul operands are bf16 (PE 1 cycle/row); accumulation is fp32 in PSUM.
Scores are computed transposed (S^T[j,i] = k^T.T @ q^T, contracting the
head's 64 dims via partition-offset slices of kT/qT) so the softmax
denominator comes free from the AV matmul via a ones-column appended to V,
and no transposes are needed anywhere. Causality is structural: only j<=i
blocks are computed; the 128x128 diagonal blocks get the (scaled) mask added.
exp() skips max-subtraction (scores are ~N(0,1); fp32 exp is safe and masked
entries underflow to exactly 0). The two diagonal-band score tiles share one
PSUM tile and one exp instruction per pair (Act instruction overhead is the
attention-phase limiter). All copies and DMA issues are kept OFF the Act
engine except the staggered mid-kernel loads.
"""

import sys

for p in ("/opt/trn_rl_repo", "/root/.axon_site/_ro/trn_rl_repo"):
    if p not in sys.path:
        sys.path.insert(0, p)

import ml_dtypes
import numpy as np

B, S, DIM, H, HD = 2, 2048, 1024, 16, 64
NCORES = 8
HG = 4  # heads per core
OG = HG * HD  # 256 output dims per core
NB = S // 512  # 4 i-blocks of 512
NJ = S // 128  # 16 j-tiles of 128

_CACHE = {}


def _build():
    import concourse.tile as tile
    from concourse import bacc, mybir

    f32 = mybir.dt.float32
    bf16 = mybir.dt.bfloat16
    Exp = mybir.ActivationFunctionType.Exp

    nc = bacc.Bacc("TRN2", target_bir_lowering=False, debug=False, num_devices=NCORES)

    # inputs, pre-packed on host (see _in_maps for layouts)
    xc = nc.dram_tensor("xc", [128, NB, 8, 512], bf16, kind="ExternalInput")
    wkq = nc.dram_tensor("wkq", [128, 4, 8, 128], bf16, kind="ExternalInput")
    wv2 = nc.dram_tensor("wv2", [128, 8, 256], bf16, kind="ExternalInput")
    woT = nc.dram_tensor("woT", [OG, DIM], bf16, kind="ExternalInput")
    cmask8 = nc.dram_tensor("cmask8", [128, 128], f32, kind="ExternalInput")
    y = nc.dram_tensor("y", [S, DIM], bf16, kind="ExternalOutput")

    woT_r = woT.ap().rearrange("(t p) e -> t p e", p=128)  # [2,128,1024]
    y_r = y.ap().rearrange("(t p) e -> t p e", p=128)  # [16,128,1024]

    with tile.TileContext(nc) as tc:
        with (
            tc.tile_pool(name="persist", bufs=1) as pp,
            tc.tile_pool(name="work", bufs=4) as wp,
            tc.tile_pool(name="psum", bufs=4, space="PSUM") as ps,
        ):
            # ---- persistent SBUF tiles -------------------------------------
            xsb = [
                pp.tile([128, 8, 512], bf16, tag=f"x{n}", name=f"x{n}")
                for n in range(NB)
            ]
            wkqt = pp.tile([128, 4, 8, 128], bf16, tag="wkq")
            wvt = pp.tile([128, 8, 256], bf16, tag="wv")
            wot = [pp.tile([128, DIM], bf16, tag=f"wo{i}", name=f"wo{i}") for i in range(2)]
            cm2 = pp.tile([128, 2, 128], f32, tag="cm2")
            kT = [pp.tile([128, S], bf16, tag=f"kT{i}", name=f"kT{i}") for i in range(2)]
            qT = [pp.tile([128, S], bf16, tag=f"qT{i}", name=f"qT{i}") for i in range(2)]
            vv = [pp.tile([128, HG, HD + 1], bf16, tag=f"vv{i}", name=f"vv{i}") for i in range(NJ)]
            zT = [pp.tile([128, S], bf16, tag=f"zT{i}", name=f"zT{i}") for i in range(2)]
            ones1 = pp.tile([1, 64], bf16, tag="ones1")
            scratch = pp.tile([128, 512], bf16, tag="scratch")

            sq, gq = nc.sync, nc.gpsimd

            # ---- upfront input DMAs (priority set: blocks 0/1 + weights) ---
            sq.dma_start(out=wkqt[:, 0], in_=wkq.ap()[:, 0])  # k m=0
            gq.dma_start(out=wkqt[:, 1], in_=wkq.ap()[:, 1])  # k m=1
            sq.dma_start(out=xsb[0][:, 0:4, :], in_=xc.ap()[:, 0, 0:4, :])
            gq.dma_start(out=xsb[0][:, 4:8, :], in_=xc.ap()[:, 0, 4:8, :])
            sq.dma_start(out=wkqt[:, 2], in_=wkq.ap()[:, 2])  # q m=0
            gq.dma_start(out=wkqt[:, 3], in_=wkq.ap()[:, 3])  # q m=1
            gq.dma_start(out=wvt, in_=wv2.ap())
            sq.dma_start(out=cm2[:, 0, :], in_=cmask8.ap())
            gq.dma_start(out=cm2[:, 1, :], in_=cmask8.ap())
            nc.vector.memset(ones1, 1.0)

            # ---- deferred softmax normalization ----------------------------
            pending = []

            def flush_norm():
                while pending:
                    pending.pop(0)()

            def mk_norm(m, po, n, psz, dn):
                def go():
                    psb2 = ps.tile([128, 1024], f32, tag="s2", bufs=3, name="psb2")
                    nc.tensor.matmul(
                        psb2[0:64, 0:512], ones1, dn, start=True, stop=True
                    )
                    rc = wp.tile([64, 512], f32, tag="rc", bufs=3, name="rc")
                    nc.vector.reciprocal_approx_fast(rc, psb2[0:64, 0:512])
                    nc.vector.tensor_mul(
                        zT[m][po : po + 64, n * 512 : (n + 1) * 512],
                        psz[0:64, :],
                        rc,
                    )
                return go

            # ---- projection units for one block (8 closures) ---------------
            def proj_units(n):
                xn = xsb[n]
                units = []

                def mk_kq(g):
                    # g: 0 = k m0, 1 = k m1, 2 = q m0, 3 = q m1
                    def go():
                        dst = kT[g % 2] if g < 2 else qT[g % 2]
                        acc2 = ps.tile([128, 1024], f32, tag="s2", bufs=3, name="acc2")
                        acc = acc2[:, 0:512]
                        for e in range(8):
                            nc.tensor.matmul(
                                acc,
                                wkqt[:, g, e, :],
                                xn[:, e, :],
                                start=(e == 0),
                                stop=(e == 7),
                            )
                        nc.vector.tensor_copy(dst[:, n * 512 : (n + 1) * 512], acc)
                    return go

                def mk_v(si):
                    def go():
                        s = 4 * n + si
                        acc2 = ps.tile([128, 1024], f32, tag="s2", bufs=3, name="acc2v")
                        acc = acc2[:, 0:256]
                        for e in range(8):
                            nc.tensor.matmul(
                                acc,
                                xn[:, e, si * 128 : (si + 1) * 128],
                                wvt[:, e, :],
                                start=(e == 0),
                                stop=(e == 7),
                            )
                        if n == 1:
                            # fired during block 0, where Act is near-idle
                            # and the DVE backlog gates the next streams
                            nc.scalar.copy(
                                vv[s][:, :, 0:HD],
                                acc.rearrange("p (h d) -> p h d", h=HG),
                            )
                        else:
                            nc.vector.tensor_copy(
                                vv[s][:, :, 0:HD],
                                acc.rearrange("p (h d) -> p h d", h=HG),
                            )
                        nc.vector.memset(vv[s][:, :, HD : HD + 1], 1.0)
                    return go

                for g in range(4):
                    units.append(mk_kq(g))
                for si in range(4):
                    units.append(mk_v(si))
                return units

            # ---- attention stream (head h, i-block n) ----------------------
            # carry: closures handed from the previous stream (its last AV +
            # denominator copy), run after this stream's first QK so the PE
            # never waits on the previous stream's final exp.
            carry = []

            def run_carry():
                while carry:
                    carry.pop(0)()

            def stream(h, n, inject, mid_cb=None, final_tail=None):
                m, po = divmod(h, 2)
                po *= 64
                kTh = kT[m][po : po + 64, :]
                qTh = qT[m][po : po + 64, :]
                ib0 = n * 512
                psz = ps.tile([65, 512], f32, tag="z", bufs=2, name="psz")
                nplain = 4 * n
                units = []  # list of (qk_closure, av_closure)

                def mk_pair(jb):
                    ex_box = []

                    def qk():
                        pss = ps.tile([128, 1024], f32, tag="s2", bufs=3, name="pss")
                        for u in range(2):
                            nc.tensor.matmul(
                                pss[:, u * 512 : (u + 1) * 512],
                                kTh[:, (jb + u) * 128 : (jb + u + 1) * 128],
                                qTh[:, ib0 : ib0 + 512],
                                start=True,
                                stop=True,
                            )
                        ex = wp.tile([128, 1024], bf16, tag="ex", name="ex", bufs=6)
                        nc.scalar.activation(ex, pss, Exp, scale=0.125)
                        ex_box.append(ex)

                    def av():
                        ex = ex_box[0]
                        for u in range(2):
                            nc.tensor.matmul(
                                psz,
                                vv[jb + u][:, h, :],
                                ex[:, u * 512 : (u + 1) * 512],
                                start=(jb + u == 0),
                                stop=False,
                            )
                    return qk, av

                def mk_band_a():
                    jb = nplain
                    ex_box = []

                    def qk():
                        A = ps.tile([128, 1024], f32, tag="s2", bufs=3, name="A")
                        nc.tensor.matmul(
                            A[:, 0:512],
                            kTh[:, jb * 128 : (jb + 1) * 128],
                            qTh[:, ib0 : ib0 + 512],
                            start=True,
                            stop=True,
                        )
                        nc.tensor.matmul(
                            A[:, 512:896],
                            kTh[:, (jb + 1) * 128 : (jb + 2) * 128],
                            qTh[:, ib0 + 128 : ib0 + 512],
                            start=True,
                            stop=True,
                        )
                        Av = A.rearrange("p (t c) -> p t c", c=512)[:, :, 0:128]
                        nc.vector.tensor_add(Av, Av, cm2)
                        exA = wp.tile([128, 896], bf16, tag="exb", name="exA", bufs=6)
                        nc.scalar.activation(exA, A[:, 0:896], Exp, scale=0.125)
                        ex_box.append(exA)

                    def av():
                        exA = ex_box[0]
                        nc.tensor.matmul(
                            psz,
                            vv[jb][:, h, :],
                            exA[:, 0:512],
                            start=(jb == 0),
                            stop=False,
                        )
                        nc.tensor.matmul(
                            psz[:, 128:512],
                            vv[jb + 1][:, h, :],
                            exA[:, 512:896],
                            start=False,
                            stop=False,
                        )
                    return qk, av

                def mk_band_b():
                    jb = nplain + 2
                    ex_box = []

                    def qk():
                        Bt = ps.tile([128, 1024], f32, tag="s2", bufs=3, name="Bt")
                        nc.tensor.matmul(
                            Bt[:, 0:256],
                            kTh[:, jb * 128 : (jb + 1) * 128],
                            qTh[:, ib0 + 256 : ib0 + 512],
                            start=True,
                            stop=True,
                        )
                        nc.tensor.matmul(
                            Bt[:, 256:384],
                            kTh[:, (jb + 1) * 128 : (jb + 2) * 128],
                            qTh[:, ib0 + 384 : ib0 + 512],
                            start=True,
                            stop=True,
                        )
                        Bv = Bt[:, 0:512].rearrange("p (t c) -> p t c", c=256)[
                            :, :, 0:128
                        ]
                        nc.vector.tensor_add(Bv, Bv, cm2)
                        exB = wp.tile([128, 896], bf16, tag="exb", name="exB", bufs=6)
                        nc.scalar.activation(exB[:, 0:384], Bt[:, 0:384], Exp, scale=0.125)
                        ex_box.append(exB)

                    def av():
                        exB = ex_box[0]
                        nc.tensor.matmul(
                            psz[:, 256:512],
                            vv[jb][:, h, :],
                            exB[:, 0:256],
                            start=False,
                            stop=False,
                        )
                        nc.tensor.matmul(
                            psz[:, 384:512],
                            vv[jb + 1][:, h, :],
                            exB[:, 256:384],
                            start=False,
                            stop=True,
                        )
                    return qk, av

                for jb in range(0, nplain, 2):
                    units.append(mk_pair(jb))
                units.append(mk_band_a())
                units.append(mk_band_b())

                flush_at = min(3, len(units) - 1)
                mid_at = min(flush_at + 1, len(units) - 1)
                for ui, (qk, _) in enumerate(units):
                    qk()
                    if ui == 0:
                        run_carry()
                    elif ui >= 3:
                        units[ui - 3][1]()
                    inject()
                    if ui == flush_at:
                        flush_norm()
                    if ui == mid_at and mid_cb is not None:
                        mid_cb()

                if final_tail is None:
                    def tail():
                        for k in range(min(3, len(units)), 0, -1):
                            units[-k][1]()
                        dn = wp.tile([1, 512], bf16, tag="dn", bufs=3, name="dn")
                        nc.vector.tensor_copy(dn, psz[64:65, :])
                        pending.append(mk_norm(m, po, n, psz, dn))

                    carry.append(tail)
                    return

                # last stream of the kernel: psz[:, 0:256] is final after band
                # A's AV (band B only touches cols >= 256), so normalize that
                # half early and emit the first two output tiles while band
                # B's AV and the second half-norm still run.
                def half_norm(c0, c1):
                    dnh = wp.tile([1, 512], bf16, tag="dn", bufs=3, name="dnh")
                    nc.scalar.copy(dnh[:, c0:c1], psz[64:65, c0:c1])
                    psbh = ps.tile([128, 1024], f32, tag="s2", bufs=3, name="psbh")
                    nc.tensor.matmul(
                        psbh[0:64, c0:c1], ones1, dnh[:, c0:c1], start=True, stop=True
                    )
                    rch = wp.tile([64, 512], f32, tag="rc", bufs=3, name="rch")
                    nc.vector.reciprocal_approx_fast(rch[:, c0:c1], psbh[0:64, c0:c1])
                    nc.vector.tensor_mul(
                        zT[m][po : po + 64, ib0 + c0 : ib0 + c1],
                        psz[0:64, c0:c1],
                        rch[:, c0:c1],
                    )

                pre, mid2, post = final_tail
                for k in range(min(3, len(units)), 1, -1):
                    units[-k][1]()
                for cb in pre:
                    cb()
                half_norm(0, 256)
                for cb in mid2:
                    cb()
                units[-1][1]()
                half_norm(256, 512)
                for cb in post:
                    cb()

            # ---- output projection for one 128-row s-tile ------------------
            def oproj(s):
                psy2 = ps.tile([128, 1024], f32, tag="s2", bufs=3, name="psy2")
                for n2 in range(2):
                    for kk in range(2):
                        nc.tensor.matmul(
                            psy2[:, n2 * 512 : (n2 + 1) * 512],
                            zT[kk][:, s * 128 : (s + 1) * 128],
                            wot[kk][:, n2 * 512 : (n2 + 1) * 512],
                            start=(kk == 0),
                            stop=(kk == 1),
                        )
                ysb = wp.tile([128, DIM], bf16, tag="ysb", bufs=3, name="ysb")
                if s < 8:
                    nc.scalar.copy(ysb[:, 0:512], psy2[:, 0:512])
                else:
                    nc.vector.tensor_copy(ysb[:, 0:512], psy2[:, 0:512])
                sq.dma_start(out=y_r[s][:, 0:512], in_=ysb[:, 0:512])
                nc.vector.tensor_copy(ysb[:, 512:1024], psy2[:, 512:1024])
                sq.dma_start(out=y_r[s][:, 512:1024], in_=ysb[:, 512:1024])

            # ---- main pipelined loop ---------------------------------------
            def act_dma(dst, src):
                def go():
                    nc.scalar.dma_start(out=dst, in_=src)
                return go

            nc.vector.memset(scratch, 0.0)
            # warmup: keep the PE streaming while the first inputs land so
            # the HAM clock boost engages before real work starts (a cold PE
            # runs at half clock for its first ~3us of busy time)
            for w in range(9):
                wps = ps.tile([128, 1024], f32, tag="s2", bufs=3, name="wps")
                for r in range(4):
                    nc.tensor.matmul(
                        wps[:, 0:512], scratch[:, 0:128], scratch,
                        start=(r == 0), stop=(r == 3),
                    )

            for u in proj_units(0):
                u()

            opq = []
            for n in range(NB):
                # aux DMA issues (Act queue, delayed by its in-order position)
                # + projection units of block n+1, injected into this block's
                # attention streams
                nxt = []
                if n == 0:
                    nxt.append(act_dma(xsb[1][:, 0:4, :], xc.ap()[:, 1, 0:4, :]))
                    nxt.append(act_dma(xsb[1][:, 4:8, :], xc.ap()[:, 1, 4:8, :]))
                    nxt.append(act_dma(wot[0], woT_r[0]))
                    nxt.append(act_dma(wot[1], woT_r[1]))
                if n + 2 < NB:
                    nxt.append(
                        act_dma(xsb[n + 2], xc.ap()[:, n + 2])
                    )
                if n + 1 < NB:
                    nxt.extend(proj_units(n + 1))
                n_units = 4 * (2 * n + 2)
                frac = [0.0]

                def inject(nxt=nxt, n_units=n_units, frac=frac, total=len(nxt)):
                    frac[0] += total / n_units
                    while nxt and frac[0] >= 1.0:
                        frac[0] -= 1.0
                        nxt.pop(0)()

                for h in range(HG):
                    mid = None
                    if opq and (n > 1 or h > 0):
                        s_out = opq.pop(0)
                        mid = (lambda s_out=s_out: oproj(s_out))
                    ft = None
                    if n == NB - 1 and h == HG - 1:
                        ft = (
                            [lambda: oproj(11)],
                            [lambda: oproj(12), lambda: oproj(13)],
                            [lambda: oproj(14), lambda: oproj(15)],
                        )
                    stream(h, n, inject, mid, ft)
                    if n > 0:
                        opq.append(4 * (n - 1) + h)
                while nxt:
                    nxt.pop(0)()

    nc.compile()
    return nc


def _get_nc():
    if "nc" not in _CACHE:
        _CACHE["nc"] = _build()
    return _CACHE["nc"]


def _in_maps(x, mask, wq, wk, wv, wo):
    bf = ml_dtypes.bfloat16
    cm8 = np.ascontiguousarray(8.0 * np.asarray(mask)[0, 0, :128, :128].T, np.float32)
    maps = []
    for c in range(NCORES):
        b, g = divmod(c, HG)
        sl = slice(OG * g, OG * (g + 1))
        # xc[p, n, e, c] = x[b][512n+c, 128e+p]
        xT = np.asarray(x)[b].T.astype(bf)  # [1024, 2048]
        xcm = np.ascontiguousarray(
            xT.reshape(8, 128, NB, 512).transpose(1, 2, 0, 3)
        )
        # wkq[p, g, e, :]: g0/g1 = wk m-chunks, g2/g3 = wq m-chunks
        wkT = np.asarray(wk)[sl, :].T.astype(bf)  # [1024, 256]
        wqT = np.asarray(wq)[sl, :].T.astype(bf)
        wkqm = np.ascontiguousarray(
            np.stack(
                [wkT[:, 0:128], wkT[:, 128:256], wqT[:, 0:128], wqT[:, 128:256]],
                axis=0,
            )
            .reshape(4, 8, 128, 128)
            .transpose(2, 0, 1, 3)
        )
        wvT = np.asarray(wv)[sl, :].T.astype(bf)  # [1024, 256]
        wv2m = np.ascontiguousarray(wvT.reshape(8, 128, 256).transpose(1, 0, 2))
        maps.append(
            {
                "xc": xcm,
                "wkq": wkqm,
                "wv2": wv2m,
                "woT": np.ascontiguousarray(np.asarray(wo)[:, sl].T).astype(bf),
                "cmask8": cm8,
            }
        )
    return maps


def _combine(results):
    y = np.zeros((B, S, DIM), np.float32)
    for c in range(NCORES):
        y[c // HG] += results[c]["y"].astype(np.float32)
    return y


def kernel(x, mask, wq, wk, wv, wo, **run_kwargs):
    from concourse.bass_utils import run_bass_kernel_spmd

    nc = _get_nc()
    res = run_bass_kernel_spmd(
        nc, _in_maps(x, mask, wq, wk, wv, wo), core_ids=list(range(NCORES)),
        **run_kwargs,
    )
    out = _combine(res.results)
    if run_kwargs:
        _CACHE["last_result"] = res
    return out
